# revision 1
# baseline (speedup 1.0000x reference)
"""Self-attention (SAGAN-style) Bass kernel for one TRN2 chip (8 NeuronCores).

Reference computation (B=4, H=W=64, C=256, D=32, N=H*W=4096):
    xf = x.reshape(B, N, C)
    k = xf @ Wk + bk; q = xf @ Wq + bq; v = xf @ Wv + bv
    energy = q @ k^T            # [B, N, N]
    attn = softmax(energy, -1)
    feat = attn @ v
    out = gamma * (feat @ Wo + bo) + xf

Sharding: core i handles batch b=i//2, query-row half h=i%2 (2048 rows).
k/v are computed over the full 4096 rows on every core (replicated, cheap).

Host-side exact folds (no device cost):
  - bk drops out of softmax (adds a per-row constant to energy).
  - bq enters energy as (k @ bq)[m]: fold Wk@bq as an extra column of Wk,
    paired with a constant-1 row appended to q^T (energy contraction K=33).
  - v-bias: attn rows sum to 1 so attn@(v+bv) = attn@v + bv; fold
    gamma*(bv@Wo + bo) into an extra row of Wo paired with a ones row of
    feat^T; gamma scales Wo itself.

Device pipeline per core (all layouts chosen so softmax reduces along the
matmul contraction axis and no on-device transposes are needed):
  kT_aug[33,4096] = Wk_aug^T @ x^T        qT_aug[33,2048] (row 32 = 1)
  v_aug[4096,33]  (col 32 = 1)
  per m-tile: S^T[m,q] on TensorE (float32r), exp on ScalarE (PSUM->SBUF),
  PV: U^T[33,512] += v_aug[m]^T @ expS^T[m]   (row 32 = row-sums r)
  feat^T = U^T[0:32] * (1/r) (rank-1 PE broadcast of 1/r), O = feat_aug^T^T
  @ Wo_aug, out = O + x_rows.
"""
import numpy as np
from contextlib import ExitStack

import concourse.bass as bass
import concourse.bacc as bacc
import concourse.tile as tile
from concourse import mybir
from concourse import bass_utils

F32 = mybir.dt.float32
F32R = mybir.dt.float32r

B, HH, WW, C = 4, 64, 64, 256
N = HH * WW          # 4096 key/value rows
D = 32               # head dim
NCORES = 8
QSH = N // 2         # 2048 query rows per core
SBW = 512            # superblock width (q columns per S^T matmul)
NSB = QSH // SBW     # 4 superblocks
NMT = N // 128       # 32 m-tiles
ts = bass.ts


def build_graph():
    """Build and compile the per-core Bass graph (identical on all cores)."""
    nc = bacc.Bacc("TRN2", target_bir_lowering=False, debug=False)

    xT_d = nc.dram_tensor("xT", [C, N], F32R, kind="ExternalInput").ap()
    xr_d = nc.dram_tensor("xr", [QSH, C], F32, kind="ExternalInput").ap()
    wk_d = nc.dram_tensor("wk", [C, 33], F32R, kind="ExternalInput").ap()
    wq_d = nc.dram_tensor("wq", [C, D], F32R, kind="ExternalInput").ap()
    wv_d = nc.dram_tensor("wv", [C, D], F32R, kind="ExternalInput").ap()
    wo_d = nc.dram_tensor("wo", [33, C + 2], F32R, kind="ExternalInput").ap()
    out_d = nc.dram_tensor("out", [QSH, C], F32, kind="ExternalOutput").ap()

    with tile.TileContext(nc) as tc, ExitStack() as ctx:
        persist = ctx.enter_context(tc.tile_pool(name="persist", bufs=1))
        st_pool = ctx.enter_context(
            tc.tile_pool(name="stps", bufs=2, space="PSUM")
        )
        uT_pool = ctx.enter_context(
            tc.tile_pool(name="uTps", bufs=1, space="PSUM")
        )
        expp = ctx.enter_context(tc.tile_pool(name="expp", bufs=3))
        smallp = ctx.enter_context(tc.tile_pool(name="smallp", bufs=2))
        outp = ctx.enter_context(tc.tile_pool(name="outp", bufs=3))

        # ---- persistent SBUF tensors ----
        xT0 = persist.tile([128, N], F32R)   # x^T rows 0:128 (channels)
        xT1 = persist.tile([128, N], F32R)   # x^T rows 128:256
        xr_sb = persist.tile([128, 16 * C], F32)  # residual rows, tile t at cols 256t
        wk_sb = persist.tile([128, 66], F32R)
        wq_sb = persist.tile([128, 64], F32R)
        wv_sb = persist.tile([128, 64], F32R)
        wo_sb = persist.tile([33, C + 2], F32R)
        # kT2: m-tiles 0..15 in rows 0:33 (cols 128*g), m-tiles 16..31 in
        # rows 64:97 — lets S^T row-pack pairs (g, g+16) at row groups 0/64.
        kT_sb = persist.tile([128, N // 2], F32R)
        # qT2: rows 0:33 = qT_aug, rows 64:97 = duplicate (for row group 64)
        qT_sb = persist.tile([128, QSH], F32R)
        v_sb = persist.tile([128, 33 * NMT], F32R)

        # ---- input DMAs ----
        nc.sync.dma_start(wk_sb[:, 0:33], wk_d[0:128, :])
        nc.sync.dma_start(wk_sb[:, 33:66], wk_d[128:256, :])
        nc.sync.dma_start(wq_sb[:, 0:32], wq_d[0:128, :])
        nc.sync.dma_start(wq_sb[:, 32:64], wq_d[128:256, :])
        nc.sync.dma_start(wv_sb[:, 0:32], wv_d[0:128, :])
        nc.sync.dma_start(wv_sb[:, 32:64], wv_d[128:256, :])
        nc.sync.dma_start(wo_sb[:], wo_d)
        nc.sync.dma_start(xT0[:], xT_d[0:128, :])
        nc.sync.dma_start(xT1[:], xT_d[128:256, :])
        for t in range(16):
            nc.sync.dma_start(xr_sb[:, ts(t, C)], xr_d[ts(t, 128), :])

        nc.vector.memset(qT_sb[32:33, :].bitcast(F32), 1.0)
        nc.vector.memset(v_sb[:].bitcast(F32), 1.0)

        # ---- projections ----
        # qT rows 0:32 = Wq^T @ xT (own-half columns of xT are 0:QSH)
        for nt in range(QSH // SBW):
            pq = st_pool.tile([32, SBW], F32, tag="st")
            nc.tensor.matmul(pq[:], wq_sb[:, 0:32], xT0[:, ts(nt, SBW)],
                             start=True, stop=False)
            nc.tensor.matmul(pq[:], wq_sb[:, 32:64], xT1[:, ts(nt, SBW)],
                             start=False, stop=True)
            nc.vector.tensor_copy(qT_sb[0:32, ts(nt, SBW)], pq[:])
        nc.vector.tensor_copy(qT_sb[64:97, :], qT_sb[0:33, :])
        # kT_aug = Wk_aug^T @ xT over all 4096 columns
        for nt in range(N // SBW):
            pk = st_pool.tile([33, SBW], F32, tag="st")
            nc.tensor.matmul(pk[:], wk_sb[:, 0:33], xT0[:, ts(nt, SBW)],
                             start=True, stop=False)
            nc.tensor.matmul(pk[:], wk_sb[:, 33:66], xT1[:, ts(nt, SBW)],
                             start=False, stop=True)
            half = N // (2 * SBW)  # 4 n-tiles per half
            if nt < half:
                nc.vector.tensor_copy(kT_sb[0:33, ts(nt, SBW)], pk[:])
            else:
                nc.vector.tensor_copy(kT_sb[64:97, ts(nt - half, SBW)], pk[:])
        # v rows: v[m, 0:32], col 32 stays 1.0 from the memset
        for j in range(NMT):
            pv = st_pool.tile([128, 32], F32, tag="st")
            nc.tensor.matmul(pv[:], xT0[:, ts(j, 128)], wv_sb[:, 0:32],
                             start=True, stop=False)
            nc.tensor.matmul(pv[:], xT1[:, ts(j, 128)], wv_sb[:, 32:64],
                             start=False, stop=True)
            nc.vector.tensor_copy(v_sb[:, 33 * j:33 * j + 32], pv[:])

        # ---- attention: m-tile pairs (g, g+16) row/col packed on the PE ----
        # S^T pair runs concurrently in row groups 0/64; PV pair runs
        # concurrently in col groups 0/64, accumulating partial U^T sums in
        # partitions 0:33 and 64:97 of one PSUM bank (summed in the tail).
        uT = []
        for s in range(NSB):
            u = uT_pool.tile([33, SBW], F32, name=f"uT{s}", tag=f"uT{s}")
            uT.append(u)

        NG = NMT // 2  # 16 m-tile pairs
        for g in range(NG):
            for s in range(NSB):
                stp = st_pool.tile([128, 1024], F32, tag="st")
                nc.tensor.matmul(stp[:, 0:SBW],
                                 kT_sb[0:33, ts(g, 128)],
                                 qT_sb[0:33, ts(s, SBW)],
                                 tile_position=(0, 0))
                nc.tensor.matmul(stp[:, SBW:1024],
                                 kT_sb[64:97, ts(g, 128)],
                                 qT_sb[64:97, ts(s, SBW)],
                                 tile_position=(64, 0))
                ex = expp.tile([128, 1024], F32R)
                nc.scalar.activation(ex[:], stp[:],
                                     mybir.ActivationFunctionType.Exp)
                nc.tensor.matmul(uT[s][:],
                                 v_sb[:, 33 * g:33 * g + 33],
                                 ex[:, 0:SBW],
                                 start=(g == 0), stop=False,
                                 skip_group_check=True)
                nc.tensor.matmul(uT[s][:],
                                 v_sb[:, 33 * (g + 16):33 * (g + 16) + 33],
                                 ex[:, SBW:1024],
                                 start=False, stop=(g == NG - 1),
                                 skip_group_check=True)

        # ---- output projection (O1 col 256 = row-sums r) + normalize ----
        # wo_aug2: rows 0:32 = gamma*Wo with a zero col 256; row 32 =
        # zeros except [32, 256] = 1, so O1[:, 256] = r. out = O1*1/r + xr'
        # (bo and bv folds are host-added to xr').
        for s in range(NSB):
            usb = smallp.tile([33, SBW], F32R, tag="usb")
            nc.vector.tensor_copy(usb[:], uT[s][:])
            for qb in range(SBW // 128):
                o_ps = st_pool.tile([128, C + 2], F32, tag="st")
                nc.tensor.matmul(o_ps[:], usb[:, ts(qb, 128)], wo_sb[:])
                recip = smallp.tile([128, 1], F32, tag="recip")
                nc.vector.reciprocal(recip[:], o_ps[:, C:C + 1])
                qi = s * (SBW // 128) + qb
                ost = outp.tile([128, C], F32)
                nc.vector.scalar_tensor_tensor(
                    ost[:], o_ps[:, 0:C], recip[:], xr_sb[:, ts(qi, C)],
                    op0=mybir.AluOpType.mult, op1=mybir.AluOpType.add,
                )
                nc.sync.dma_start(out_d[ts(qi, 128), :], ost[:])

    nc.compile()
    return nc


_NC_CACHE = None


def _get_nc():
    global _NC_CACHE
    if _NC_CACHE is None:
        _NC_CACHE = build_graph()
    return _NC_CACHE


def make_in_maps(x, Wk, bk, Wq, bq, Wv, bv, Wo, bo, gamma):
    """Host-side sharding + exact bias/gamma folding."""
    f32 = np.float32
    xf = np.ascontiguousarray(x, dtype=f32).reshape(B, N, C)
    Wk = np.asarray(Wk, dtype=f32)
    Wq = np.asarray(Wq, dtype=f32)
    Wv = np.asarray(Wv, dtype=f32)
    Wo = np.asarray(Wo, dtype=f32)
    bk = np.asarray(bk, dtype=f32)
    bq = np.asarray(bq, dtype=f32)
    bv = np.asarray(bv, dtype=f32)
    bo = np.asarray(bo, dtype=f32)
    g = np.asarray(gamma, dtype=f32)[0]

    wk_aug = np.concatenate([Wk, (Wk @ bq)[:, None]], axis=1)  # [C, 33]
    wo_aug = np.zeros((33, C + 2), dtype=f32)
    wo_aug[0:32, 0:C] = g * Wo
    wo_aug[32, C] = 1.0
    xr_bias = (g * (bv @ Wo + bo)).astype(f32)  # folded into the residual

    in_maps = []
    for i in range(NCORES):
        b, h = divmod(i, 2)
        own = xf[b, h * QSH:(h + 1) * QSH]
        other = xf[b, (1 - h) * QSH:(2 - h) * QSH]
        xT = np.ascontiguousarray(
            np.concatenate([own, other], axis=0).T
        )  # [C, N], own-half columns first
        in_maps.append({
            "xT": xT,
            "xr": np.ascontiguousarray(own + xr_bias),
            "wk": np.ascontiguousarray(wk_aug),
            "wq": Wq,
            "wv": Wv,
            "wo": np.ascontiguousarray(wo_aug),
        })
    return in_maps


def gather_out(results, x_dtype):
    out = np.empty((B, N, C), dtype=np.float32)
    for i in range(NCORES):
        b, h = divmod(i, 2)
        out[b, h * QSH:(h + 1) * QSH] = results[i]["out"]
    return out.reshape(B, HH, WW, C).astype(x_dtype, copy=False)


def kernel(x, Wk, bk, Wq, bq, Wv, bv, Wo, bo, gamma, **run_kwargs):
    nc = _get_nc()
    in_maps = make_in_maps(x, Wk, bk, Wq, bq, Wv, bv, Wo, bo, gamma)
    res = bass_utils.run_bass_kernel_spmd(
        nc, in_maps, core_ids=list(range(NCORES)), **run_kwargs
    )
    out = gather_out(res.results, np.asarray(x).dtype)
    if run_kwargs:
        return out, res
    return out



# revision 4
# speedup vs baseline: 1.0665x; 1.0665x over previous
"""Self-attention (SAGAN-style) Bass kernel for one TRN2 chip (8 NeuronCores).

Reference computation (B=4, H=W=64, C=256, D=32, N=H*W=4096):
    xf = x.reshape(B, N, C)
    k = xf @ Wk + bk; q = xf @ Wq + bq; v = xf @ Wv + bv
    energy = q @ k^T            # [B, N, N]
    attn = softmax(energy, -1)
    feat = attn @ v
    out = gamma * (feat @ Wo + bo) + xf

Sharding: core i handles batch b=i//2, query-row half h=i%2 (2048 rows).
k/v are computed over the full 4096 rows on every core (replicated, cheap).

Host-side exact folds (no device cost):
  - bk drops out of softmax (adds a per-row constant to energy).
  - v-bias: attn rows sum to 1 so attn@(v+bv) = attn@v + bv; fold
    gamma*(bv@Wo + bo) into the residual rows; gamma scales Wo itself.
  - bq enters energy as c_m = (k_m . bq), i.e. exp(S + c_m) =
    exp(S)*exp(c_m): a per-KEY scale that can be folded into v and the
    softmax-denominator ones column.  c_m = x_m . (Wk@bq) is computed on
    device as an extra wv column (the v projection already produces
    m-rows on partitions), so no transposes are needed.  When bq == 0
    (this problem's inputs) the whole path is compiled out.

Device pipeline per core, all PE operands bf16 (f32 PSUM accumulate):
  prologue: chunked xT DMA overlapped with q/k/v projections.
    qT [32,2048] replicated to partition groups 0/32/64.
    kT4: m-tile j at partition group 32*(slot) and col 128*(window).
    v4 [128, 33*32]: col 32 of each block = 1 (softmax denominator).
  main loop (s = 4 superblocks of 512 q-cols; 11 windows of 3|2 m-tiles):
    S^T: per window, one matmul per m-tile, 4x-row-packed
         (tile_position (32r,0), K=32) -> PSUM [128, 512*nw]
    exp: single ScalarE activation [128, 1536] PSUM->SBUF bf16
    PV:  2x-packed 64x64 tiles (0,0)+(64,64): U partials accumulate in
         partitions 0:33 / 64:97 of ONE psum bank across all 32 m-tiles
         (row 32 = column sums r, via the v4 ones column).
  epilogue per superblock: usb = (U0+U1) bf16, out-proj vs Wo_aug
  (col 256 carries r through), out = O*(1/r) + xr  (residual, f32).
ScalarE exp (~67us) is the designed critical path; PE work fits under it.
"""
import numpy as np
from contextlib import ExitStack

import concourse.bass as bass
import concourse.bacc as bacc
import concourse.tile as tile
from concourse import mybir
from concourse import bass_utils

F32 = mybir.dt.float32
BF16 = mybir.dt.bfloat16

B, HH, WW, C = 4, 64, 64, 256
N = HH * WW          # 4096 key/value rows
D = 32               # head dim
NCORES = 8
QSH = N // 2         # 2048 query rows per core
SBW = 512            # superblock width (q columns)
NSB = QSH // SBW     # 4 superblocks
NMT = N // 128       # 32 m-tiles
NW = [3] * 10 + [2]  # m-tiles per window (sum = 32)
ts = bass.ts

# m-tile j -> (window w, slot r); slot r uses PE row group 32r
_J2WR = {}
_jb = 0
for _w, _nw in enumerate(NW):
    for _r in range(_nw):
        _J2WR[_jb + _r] = (_w, _r)
    _jb += _nw


def build_graph(with_bq=False):
    """Build and compile the per-core Bass graph (identical on all cores)."""
    nc = bacc.Bacc("TRN2", target_bir_lowering=False, debug=False)

    xT_d = nc.dram_tensor("xT", [C, N], BF16, kind="ExternalInput").ap()
    xr_d = nc.dram_tensor("xr", [QSH, C], F32, kind="ExternalInput").ap()
    wk_d = nc.dram_tensor("wk", [C, D], BF16, kind="ExternalInput").ap()
    wq_d = nc.dram_tensor("wq", [C, D], BF16, kind="ExternalInput").ap()
    NV = 33 if with_bq else 32  # wv gets an extra Wk@bq column when bq != 0
    wv_d = nc.dram_tensor("wv", [C, NV], BF16, kind="ExternalInput").ap()
    wo_d = nc.dram_tensor("wo", [33, C + 2], BF16, kind="ExternalInput").ap()
    out_d = nc.dram_tensor("out", [QSH, C], F32, kind="ExternalOutput").ap()

    ExpF = mybir.ActivationFunctionType.Exp
    Amul = mybir.AluOpType.mult
    Aadd = mybir.AluOpType.add

    with tile.TileContext(nc) as tc, ExitStack() as ctx:
        persist = ctx.enter_context(tc.tile_pool(name="persist", bufs=1))
        stp = ctx.enter_context(tc.tile_pool(name="stp", bufs=2, space="PSUM"))
        uTp = ctx.enter_context(tc.tile_pool(name="uTp", bufs=1, space="PSUM"))
        outp = ctx.enter_context(
            tc.tile_pool(name="outp", bufs=1, space="PSUM"))
        expp = ctx.enter_context(tc.tile_pool(name="expp", bufs=3))
        smallp = ctx.enter_context(tc.tile_pool(name="smallp", bufs=2))
        outsb = ctx.enter_context(tc.tile_pool(name="outsb", bufs=3))

        # ---- persistent SBUF tensors ----
        # xT in 4 column chunks x 2 channel halves (separate tiles so the
        # projections for chunk t only depend on chunk t's DMA).
        xt = [[persist.tile([128, 1024], BF16, name=f"xt{t}_{hf}")
               for hf in range(2)] for t in range(4)]
        xr_sb = persist.tile([128, 16 * C], F32)  # residual row-tile t at 256t
        wk_sb = persist.tile([128, 64], BF16)
        wq_sb = persist.tile([128, 64], BF16)
        wv_sb = persist.tile([128, 2 * NV], BF16)
        wo_sb = persist.tile([33, C + 2], BF16)
        qT4 = persist.tile([128, QSH], BF16)      # rows 32r:32r+32, r=0..2
        kT4 = persist.tile([128, 128 * len(NW)], BF16)
        v4 = persist.tile([128, 33 * NMT], BF16)
        dummy = persist.tile([1, 1], F32)
        if with_bq:
            kbq_sb = persist.tile([128, NMT], F32)
            ekbq = persist.tile([128, NMT], F32)

        # preload the exp table set while the prologue runs
        nc.vector.memset(dummy[:], 0.0)
        nc.scalar.activation(dummy[:], dummy[:], ExpF)

        # ---- input DMAs ----
        nc.sync.dma_start(wk_sb[:, 0:32], wk_d[0:128, :])
        nc.sync.dma_start(wk_sb[:, 32:64], wk_d[128:256, :])
        nc.sync.dma_start(wq_sb[:, 0:32], wq_d[0:128, :])
        nc.sync.dma_start(wq_sb[:, 32:64], wq_d[128:256, :])
        nc.sync.dma_start(wv_sb[:, 0:NV], wv_d[0:128, :])
        nc.sync.dma_start(wv_sb[:, NV:2 * NV], wv_d[128:256, :])
        nc.sync.dma_start(wo_sb[:], wo_d)
        for t in range(4):
            nc.sync.dma_start(xt[t][0][:], xT_d[0:128, ts(t, 1024)])
            nc.sync.dma_start(xt[t][1][:], xT_d[128:256, ts(t, 1024)])
        for t in range(16):
            nc.sync.dma_start(xr_sb[:, ts(t, C)], xr_d[ts(t, 128), :])

        if not with_bq:
            nc.vector.memset(v4[:], 1.0)  # col 32 of each block stays 1

        # ---- projections (per xT chunk t: 1024 m-columns = 8 m-tiles) ----
        for t in range(4):
            x0, x1 = xt[t]
            # q projection: only own-half chunks (cols 0:2048)
            if t < 2:
                for half in range(2):
                    pq = stp.tile([32, SBW], F32, tag="st")
                    nc.tensor.matmul(pq[:], wq_sb[:, 0:32],
                                     x0[:, ts(half, SBW)],
                                     start=True, stop=False)
                    nc.tensor.matmul(pq[:], wq_sb[:, 32:64],
                                     x1[:, ts(half, SBW)],
                                     start=False, stop=True)
                    nt = 2 * t + half
                    nc.vector.tensor_copy(qT4[0:32, ts(nt, SBW)], pq[:])
            # k projection -> kT4 grouped layout
            for half in range(2):
                pk = stp.tile([32, SBW], F32, tag="st")
                nc.tensor.matmul(pk[:], wk_sb[:, 0:32],
                                 x0[:, ts(half, SBW)],
                                 start=True, stop=False)
                nc.tensor.matmul(pk[:], wk_sb[:, 32:64],
                                 x1[:, ts(half, SBW)],
                                 start=False, stop=True)
                for jj in range(4):
                    j = 8 * t + 4 * half + jj
                    w, r = _J2WR[j]
                    nc.vector.tensor_copy(
                        kT4[32 * r:32 * r + 32, ts(w, 128)],
                        pk[:, ts(jj, 128)])
            # v projection (xT chunk stationary, wv moving)
            for jj in range(8):
                j = 8 * t + jj
                pv = stp.tile([128, NV], F32, tag="st")
                nc.tensor.matmul(pv[:], x0[:, ts(jj, 128)], wv_sb[:, 0:NV],
                                 start=True, stop=False)
                nc.tensor.matmul(pv[:], x1[:, ts(jj, 128)],
                                 wv_sb[:, NV:2 * NV],
                                 start=False, stop=True)
                nc.vector.tensor_copy(v4[:, 33 * j:33 * j + 32], pv[:, 0:32])
                if with_bq:
                    nc.vector.tensor_copy(kbq_sb[:, j:j + 1], pv[:, 32:33])
        # replicate qT to partition groups 1, 2 (PE row groups 32, 64)
        nc.vector.tensor_copy(qT4[32:64, :], qT4[0:32, :])
        nc.vector.tensor_copy(qT4[64:96, :], qT4[0:32, :])
        if with_bq:
            # exact bq handling: scale v rows (and the ones column) by
            # exp(k_m . bq) so softmax numerator/denominator pick it up
            nc.scalar.activation(ekbq[:], kbq_sb[:], ExpF)
            for j in range(NMT):
                nc.vector.tensor_scalar(
                    v4[:, 33 * j:33 * j + 32], v4[:, 33 * j:33 * j + 32],
                    ekbq[:, j:j + 1], None, op0=Amul)
                nc.vector.tensor_copy(v4[:, 33 * j + 32:33 * j + 33],
                                      ekbq[:, j:j + 1])

        # ---- attention main loop ----
        uT = uTp.tile([128, SBW], F32, name="uT", tag="uT")
        for s in range(NSB):
            jb = 0
            for w, nw in enumerate(NW):
                st = stp.tile([128, SBW * nw], F32, tag="st")
                for r in range(nw):
                    nc.tensor.matmul(st[:, ts(r, SBW)],
                                     kT4[32 * r:32 * r + 32, ts(w, 128)],
                                     qT4[32 * r:32 * r + 32, ts(s, SBW)],
                                     start=True, stop=True,
                                     tile_position=(32 * r, 0))
                ex = expp.tile([128, SBW * nw], BF16)
                nc.scalar.activation(ex[:], st[:], ExpF)
                for r in range(nw):
                    j = jb + r
                    nc.tensor.matmul(uT[0:33, :],
                                     v4[0:64, 33 * j:33 * j + 33],
                                     ex[0:64, ts(r, SBW)],
                                     start=(j == 0), stop=(j == NMT - 1),
                                     skip_group_check=True,
                                     tile_position=(0, 0))
                    nc.tensor.matmul(uT[64:97, :],
                                     v4[64:128, 33 * j:33 * j + 33],
                                     ex[64:128, ts(r, SBW)],
                                     start=(j == 0), stop=(j == NMT - 1),
                                     skip_group_check=True,
                                     tile_position=(64, 64))
                jb += nw

            # ---- output projection for superblock s ----
            # DVE binary ops need lane-aligned operands: shift the upper
            # partial down with a copy, then add.
            uhi = smallp.tile([33, SBW], F32, tag="uhi")
            nc.vector.tensor_copy(uhi[:], uT[64:97, :])
            usb = smallp.tile([33, SBW], BF16, tag="usb")
            nc.vector.tensor_tensor(usb[:], uT[0:33, :], uhi[:], op=Aadd)
            for qb in range(SBW // 128):
                o_ps = outp.tile([128, C + 2], F32, tag="op")
                nc.tensor.matmul(o_ps[:], usb[:, ts(qb, 128)], wo_sb[:],
                                 start=True, stop=True)
                recip = smallp.tile([128, 1], F32, tag="recip")
                nc.vector.reciprocal(recip[:], o_ps[:, C:C + 1])
                qi = s * (SBW // 128) + qb
                ost = outsb.tile([128, C], F32)
                nc.vector.scalar_tensor_tensor(
                    ost[:], o_ps[:, 0:C], recip[:], xr_sb[:, ts(qi, C)],
                    op0=Amul, op1=Aadd)
                nc.sync.dma_start(out_d[ts(qi, 128), :], ost[:])

    nc.compile()
    return nc


_NC_CACHE = {}


def _get_nc(with_bq=False):
    if with_bq not in _NC_CACHE:
        _NC_CACHE[with_bq] = build_graph(with_bq)
    return _NC_CACHE[with_bq]


def _bf16(a):
    import ml_dtypes
    return np.ascontiguousarray(np.asarray(a, dtype=np.float32)
                                .astype(ml_dtypes.bfloat16))


def make_in_maps(x, Wk, bk, Wq, bq, Wv, bv, Wo, bo, gamma):
    """Host-side sharding + exact bias/gamma folding."""
    f32 = np.float32
    xf = np.ascontiguousarray(x, dtype=f32).reshape(B, N, C)
    Wk = np.asarray(Wk, dtype=f32)
    Wq = np.asarray(Wq, dtype=f32)
    Wv = np.asarray(Wv, dtype=f32)
    Wo = np.asarray(Wo, dtype=f32)
    bq = np.asarray(bq, dtype=f32)
    bv = np.asarray(bv, dtype=f32)
    bo = np.asarray(bo, dtype=f32)
    g = np.asarray(gamma, dtype=f32)[0]

    with_bq = bool(np.any(bq != 0.0))
    if with_bq:
        wv_in = np.concatenate([Wv, (Wk @ bq)[:, None]], axis=1)  # [C, 33]
    else:
        wv_in = Wv
    wo_aug = np.zeros((33, C + 2), dtype=f32)
    wo_aug[0:32, 0:C] = g * Wo
    wo_aug[32, C] = 1.0
    xr_bias = (g * (bv @ Wo + bo)).astype(f32)  # folded into the residual

    wk_b = _bf16(Wk)
    wq_b = _bf16(Wq)
    wv_b = _bf16(wv_in)
    wo_b = _bf16(wo_aug)

    in_maps = []
    for i in range(NCORES):
        b, h = divmod(i, 2)
        own = xf[b, h * QSH:(h + 1) * QSH]
        other = xf[b, (1 - h) * QSH:(2 - h) * QSH]
        xT = np.concatenate([own, other], axis=0).T  # [C, N], own cols first
        in_maps.append({
            "xT": _bf16(xT),
            "xr": np.ascontiguousarray(own + xr_bias),
            "wk": wk_b,
            "wq": wq_b,
            "wv": wv_b,
            "wo": wo_b,
        })
    return in_maps, with_bq


def gather_out(results, x_dtype):
    out = np.empty((B, N, C), dtype=np.float32)
    for i in range(NCORES):
        b, h = divmod(i, 2)
        out[b, h * QSH:(h + 1) * QSH] = results[i]["out"]
    return out.reshape(B, HH, WW, C).astype(x_dtype, copy=False)


def kernel(x, Wk, bk, Wq, bq, Wv, bv, Wo, bo, gamma, **run_kwargs):
    in_maps, with_bq = make_in_maps(x, Wk, bk, Wq, bq, Wv, bv, Wo, bo, gamma)
    nc = _get_nc(with_bq)
    res = bass_utils.run_bass_kernel_spmd(
        nc, in_maps, core_ids=list(range(NCORES)), **run_kwargs
    )
    out = gather_out(res.results, np.asarray(x).dtype)
    if run_kwargs:
        return out, res
    return out


# revision 5
# speedup vs baseline: 1.5744x; 1.4763x over previous
"""Self-attention (SAGAN-style) Bass kernel for one TRN2 chip (8 NeuronCores).

Reference computation (B=4, H=W=64, C=256, D=32, N=H*W=4096):
    xf = x.reshape(B, N, C)
    k = xf @ Wk + bk; q = xf @ Wq + bq; v = xf @ Wv + bv
    energy = q @ k^T            # [B, N, N]
    attn = softmax(energy, -1)
    feat = attn @ v
    out = gamma * (feat @ Wo + bo) + xf

Sharding: core i handles batch b=i//2, query-row half h=i%2 (2048 rows).
k/v are computed over the full 4096 rows on every core (replicated, cheap).

Host-side exact folds (no device cost):
  - bk drops out of softmax (adds a per-row constant to energy).
  - v-bias: attn rows sum to 1 so attn@(v+bv) = attn@v + bv; fold
    gamma*(bv@Wo + bo) into the residual rows; gamma scales Wo itself.
  - bq enters energy as c_m = (k_m . bq), i.e. exp(S + c_m) =
    exp(S)*exp(c_m): a per-KEY scale that can be folded into v and the
    softmax-denominator ones column.  c_m = x_m . (Wk@bq) is computed on
    device as an extra wv column (the v projection already produces
    m-rows on partitions), so no transposes are needed.  When bq == 0
    (this problem's inputs) the whole path is compiled out.

Device pipeline per core, all PE operands bf16 (f32 PSUM accumulate):
  prologue: chunked xT DMA overlapped with q/k/v projections.
    qT [32,2048] replicated to partition groups 0/32/64.
    kT4: m-tile j at partition group 32*(slot) and col 128*(window).
    v4 [128, 33*32]: col 32 of each block = 1 (softmax denominator).
  main loop (s = 4 superblocks of 512 q-cols; 11 windows of 3|2 m-tiles):
    S^T: per window, one matmul per m-tile, 4x-row-packed
         (tile_position (32r,0), K=32) -> PSUM [128, 512*nw]
    exp: single ScalarE activation [128, 1536] PSUM->SBUF bf16
    PV:  2x-packed 64x64 tiles (0,0)+(64,64): U partials accumulate in
         partitions 0:33 / 64:97 of ONE psum bank across all 32 m-tiles
         (row 32 = column sums r, via the v4 ones column).
  epilogue per superblock: usb = (U0+U1) bf16, out-proj vs Wo_aug
  (col 256 carries r through), out = O*(1/r) + xr  (residual, f32).
ScalarE exp (~67us) is the designed critical path; PE work fits under it.
"""
import numpy as np
from contextlib import ExitStack

import concourse.bass as bass
import concourse.bacc as bacc
import concourse.tile as tile
from concourse import mybir
from concourse import bass_utils

F32 = mybir.dt.float32
BF16 = mybir.dt.bfloat16

B, HH, WW, C = 4, 64, 64, 256
N = HH * WW          # 4096 key/value rows
D = 32               # head dim
NCORES = 8
QSH = N // 2         # 2048 query rows per core
SBW = 512            # superblock width (q columns)
NSB = QSH // SBW     # 4 superblocks
NMT = N // 128       # 32 m-tiles
NW = [3] * 10 + [2]  # m-tiles per window (sum = 32)
ts = bass.ts

# m-tile j -> (window w, slot r); slot r uses PE row group 32r
_J2WR = {}
_jb = 0
for _w, _nw in enumerate(NW):
    for _r in range(_nw):
        _J2WR[_jb + _r] = (_w, _r)
    _jb += _nw


def build_graph(with_bq=False):
    """Build and compile the per-core Bass graph (identical on all cores)."""
    nc = bacc.Bacc("TRN2", target_bir_lowering=False, debug=False)

    xT_d = nc.dram_tensor("xT", [C, N], BF16, kind="ExternalInput").ap()
    xr_d = nc.dram_tensor("xr", [QSH, C], F32, kind="ExternalInput").ap()
    wk_d = nc.dram_tensor("wk", [C, D], BF16, kind="ExternalInput").ap()
    wq_d = nc.dram_tensor("wq", [C, D], BF16, kind="ExternalInput").ap()
    NV = 33 if with_bq else 32  # wv gets an extra Wk@bq column when bq != 0
    wv_d = nc.dram_tensor("wv", [C, NV], BF16, kind="ExternalInput").ap()
    wo_d = nc.dram_tensor("wo", [33, C + 2], BF16, kind="ExternalInput").ap()
    out_d = nc.dram_tensor("out", [QSH, C], F32, kind="ExternalOutput").ap()

    ExpF = mybir.ActivationFunctionType.Exp
    Amul = mybir.AluOpType.mult
    Aadd = mybir.AluOpType.add

    with tile.TileContext(nc) as tc, ExitStack() as ctx:
        persist = ctx.enter_context(tc.tile_pool(name="persist", bufs=1))
        stp = ctx.enter_context(tc.tile_pool(name="stp", bufs=2, space="PSUM"))
        uTp = ctx.enter_context(tc.tile_pool(name="uTp", bufs=1, space="PSUM"))
        outp = ctx.enter_context(
            tc.tile_pool(name="outp", bufs=1, space="PSUM"))
        expp = ctx.enter_context(tc.tile_pool(name="expp", bufs=3))
        smallp = ctx.enter_context(tc.tile_pool(name="smallp", bufs=2))
        outsb = ctx.enter_context(tc.tile_pool(name="outsb", bufs=3))

        # ---- persistent SBUF tensors ----
        # xT in 4 column chunks x 2 channel halves (separate tiles so the
        # projections for chunk t only depend on chunk t's DMA).
        xt = [[persist.tile([128, 1024], BF16, name=f"xt{t}_{hf}")
               for hf in range(2)] for t in range(4)]
        xr_sb = persist.tile([128, 16 * C], F32)  # residual row-tile t at 256t
        wk_sb = persist.tile([128, 64], BF16)
        wq_sb = persist.tile([128, 64], BF16)
        wv_sb = persist.tile([128, 2 * NV], BF16)
        wo_sb = persist.tile([33, C + 2], BF16)
        qT4 = persist.tile([128, QSH], BF16)      # rows 32r:32r+32, r=0..2
        kT4 = persist.tile([128, 128 * len(NW)], BF16)
        v4 = persist.tile([128, 33 * NMT], BF16)
        dummy = persist.tile([1, 1], F32)
        if with_bq:
            kbq_sb = persist.tile([128, NMT], F32)
            ekbq = persist.tile([128, NMT], F32)

        # preload the exp table set while the prologue runs
        nc.vector.memset(dummy[:], 0.0)
        nc.scalar.activation(dummy[:], dummy[:], ExpF)

        # ---- input DMAs ----
        nc.sync.dma_start(wk_sb[:, 0:32], wk_d[0:128, :])
        nc.sync.dma_start(wk_sb[:, 32:64], wk_d[128:256, :])
        nc.sync.dma_start(wq_sb[:, 0:32], wq_d[0:128, :])
        nc.sync.dma_start(wq_sb[:, 32:64], wq_d[128:256, :])
        nc.sync.dma_start(wv_sb[:, 0:NV], wv_d[0:128, :])
        nc.sync.dma_start(wv_sb[:, NV:2 * NV], wv_d[128:256, :])
        nc.sync.dma_start(wo_sb[:], wo_d)
        for t in range(4):
            nc.sync.dma_start(xt[t][0][:], xT_d[0:128, ts(t, 1024)])
            nc.sync.dma_start(xt[t][1][:], xT_d[128:256, ts(t, 1024)])
        for t in range(16):
            nc.sync.dma_start(xr_sb[:, ts(t, C)], xr_d[ts(t, 128), :])

        if not with_bq:
            nc.vector.memset(v4[:], 1.0)  # col 32 of each block stays 1

        # ---- projections (per xT chunk t: 1024 m-columns = 8 m-tiles) ----
        for t in range(4):
            x0, x1 = xt[t]
            # q projection: only own-half chunks (cols 0:2048)
            if t < 2:
                for half in range(2):
                    pq = stp.tile([32, SBW], F32, tag="st")
                    nc.tensor.matmul(pq[:], wq_sb[:, 0:32],
                                     x0[:, ts(half, SBW)],
                                     start=True, stop=False)
                    nc.tensor.matmul(pq[:], wq_sb[:, 32:64],
                                     x1[:, ts(half, SBW)],
                                     start=False, stop=True)
                    nt = 2 * t + half
                    nc.vector.tensor_copy(qT4[0:32, ts(nt, SBW)], pq[:])
            # k projection -> kT4 grouped layout
            for half in range(2):
                pk = stp.tile([32, SBW], F32, tag="st")
                nc.tensor.matmul(pk[:], wk_sb[:, 0:32],
                                 x0[:, ts(half, SBW)],
                                 start=True, stop=False)
                nc.tensor.matmul(pk[:], wk_sb[:, 32:64],
                                 x1[:, ts(half, SBW)],
                                 start=False, stop=True)
                for jj in range(4):
                    j = 8 * t + 4 * half + jj
                    w, r = _J2WR[j]
                    nc.vector.tensor_copy(
                        kT4[32 * r:32 * r + 32, ts(w, 128)],
                        pk[:, ts(jj, 128)])
            # v projection (xT chunk stationary, wv moving)
            for jj in range(8):
                j = 8 * t + jj
                pv = stp.tile([128, NV], F32, tag="st")
                nc.tensor.matmul(pv[:], x0[:, ts(jj, 128)], wv_sb[:, 0:NV],
                                 start=True, stop=False)
                nc.tensor.matmul(pv[:], x1[:, ts(jj, 128)],
                                 wv_sb[:, NV:2 * NV],
                                 start=False, stop=True)
                nc.vector.tensor_copy(v4[:, 33 * j:33 * j + 32], pv[:, 0:32])
                if with_bq:
                    nc.vector.tensor_copy(kbq_sb[:, j:j + 1], pv[:, 32:33])
        # replicate qT to partition groups 1, 2 (PE row groups 32, 64)
        nc.vector.tensor_copy(qT4[32:64, :], qT4[0:32, :])
        nc.vector.tensor_copy(qT4[64:96, :], qT4[0:32, :])
        if with_bq:
            # exact bq handling: scale v rows (and the ones column) by
            # exp(k_m . bq) so softmax numerator/denominator pick it up
            nc.scalar.activation(ekbq[:], kbq_sb[:], ExpF)
            for j in range(NMT):
                nc.vector.tensor_scalar(
                    v4[:, 33 * j:33 * j + 32], v4[:, 33 * j:33 * j + 32],
                    ekbq[:, j:j + 1], None, op0=Amul)
                nc.vector.tensor_copy(v4[:, 33 * j + 32:33 * j + 33],
                                      ekbq[:, j:j + 1])

        # ---- attention main loop ----
        # Software-pipelined issue order: the PE queue is FIFO, so S^T of
        # window i+2 is issued BEFORE PV of window i — PV(i) waits ~1.5us
        # on exp(i) and everything behind it in the queue would stall.
        # PV runs as solo K=128 matmuls: solo bf16 matmuls stream 2
        # cols/cycle (double-pumped), while concurrently row-packed ones
        # drop to 1 col/cycle, so packing loses for full-K PV.
        uT = uTp.tile([128, SBW], F32, name="uT", tag="uT")
        seq = [(s, w) for s in range(NSB) for w in range(len(NW))]
        jstart = {w: sum(NW[:w]) for w in range(len(NW))}

        def issue_st(s, w):
            nw = NW[w]
            st = stp.tile([128, SBW * nw], F32, tag="st")
            for r in range(nw):
                nc.tensor.matmul(st[:, ts(r, SBW)],
                                 kT4[32 * r:32 * r + 32, ts(w, 128)],
                                 qT4[32 * r:32 * r + 32, ts(s, SBW)],
                                 start=True, stop=True,
                                 tile_position=(32 * r, 0))
            return st

        sts = {0: issue_st(*seq[0]), 1: issue_st(*seq[1])}
        for i, (s, w) in enumerate(seq):
            nw = NW[w]
            st = sts.pop(i)
            ex = expp.tile([128, SBW * nw], BF16)
            nc.scalar.activation(ex[:], st[:], ExpF)
            if i + 2 < len(seq):
                sts[i + 2] = issue_st(*seq[i + 2])
            for r in range(nw):
                j = jstart[w] + r
                nc.tensor.matmul(uT[0:33, :], v4[:, 33 * j:33 * j + 33],
                                 ex[:, ts(r, SBW)],
                                 start=(j == 0), stop=(j == NMT - 1),
                                 skip_group_check=True)
            if w != len(NW) - 1:
                continue

            # ---- output projection for superblock s ----
            usb = smallp.tile([33, SBW], BF16, tag="usb")
            nc.vector.tensor_copy(usb[:], uT[0:33, :])
            for qb in range(SBW // 128):
                o_ps = outp.tile([128, C + 2], F32, tag="op")
                nc.tensor.matmul(o_ps[:], usb[:, ts(qb, 128)], wo_sb[:],
                                 start=True, stop=True)
                recip = smallp.tile([128, 1], F32, tag="recip")
                nc.vector.reciprocal(recip[:], o_ps[:, C:C + 1])
                qi = s * (SBW // 128) + qb
                ost = outsb.tile([128, C], F32)
                nc.vector.scalar_tensor_tensor(
                    ost[:], o_ps[:, 0:C], recip[:], xr_sb[:, ts(qi, C)],
                    op0=Amul, op1=Aadd)
                nc.sync.dma_start(out_d[ts(qi, 128), :], ost[:])

    nc.compile()
    return nc


_NC_CACHE = {}


def _get_nc(with_bq=False):
    if with_bq not in _NC_CACHE:
        _NC_CACHE[with_bq] = build_graph(with_bq)
    return _NC_CACHE[with_bq]


def _bf16(a):
    import ml_dtypes
    return np.ascontiguousarray(np.asarray(a, dtype=np.float32)
                                .astype(ml_dtypes.bfloat16))


def make_in_maps(x, Wk, bk, Wq, bq, Wv, bv, Wo, bo, gamma):
    """Host-side sharding + exact bias/gamma folding."""
    f32 = np.float32
    xf = np.ascontiguousarray(x, dtype=f32).reshape(B, N, C)
    Wk = np.asarray(Wk, dtype=f32)
    Wq = np.asarray(Wq, dtype=f32)
    Wv = np.asarray(Wv, dtype=f32)
    Wo = np.asarray(Wo, dtype=f32)
    bq = np.asarray(bq, dtype=f32)
    bv = np.asarray(bv, dtype=f32)
    bo = np.asarray(bo, dtype=f32)
    g = np.asarray(gamma, dtype=f32)[0]

    with_bq = bool(np.any(bq != 0.0))
    if with_bq:
        wv_in = np.concatenate([Wv, (Wk @ bq)[:, None]], axis=1)  # [C, 33]
    else:
        wv_in = Wv
    wo_aug = np.zeros((33, C + 2), dtype=f32)
    wo_aug[0:32, 0:C] = g * Wo
    wo_aug[32, C] = 1.0
    xr_bias = (g * (bv @ Wo + bo)).astype(f32)  # folded into the residual

    wk_b = _bf16(Wk)
    wq_b = _bf16(Wq)
    wv_b = _bf16(wv_in)
    wo_b = _bf16(wo_aug)

    in_maps = []
    for i in range(NCORES):
        b, h = divmod(i, 2)
        own = xf[b, h * QSH:(h + 1) * QSH]
        other = xf[b, (1 - h) * QSH:(2 - h) * QSH]
        xT = np.concatenate([own, other], axis=0).T  # [C, N], own cols first
        in_maps.append({
            "xT": _bf16(xT),
            "xr": np.ascontiguousarray(own + xr_bias),
            "wk": wk_b,
            "wq": wq_b,
            "wv": wv_b,
            "wo": wo_b,
        })
    return in_maps, with_bq


def gather_out(results, x_dtype):
    out = np.empty((B, N, C), dtype=np.float32)
    for i in range(NCORES):
        b, h = divmod(i, 2)
        out[b, h * QSH:(h + 1) * QSH] = results[i]["out"]
    return out.reshape(B, HH, WW, C).astype(x_dtype, copy=False)


def kernel(x, Wk, bk, Wq, bq, Wv, bv, Wo, bo, gamma, **run_kwargs):
    in_maps, with_bq = make_in_maps(x, Wk, bk, Wq, bq, Wv, bv, Wo, bo, gamma)
    nc = _get_nc(with_bq)
    res = bass_utils.run_bass_kernel_spmd(
        nc, in_maps, core_ids=list(range(NCORES)), **run_kwargs
    )
    out = gather_out(res.results, np.asarray(x).dtype)
    if run_kwargs:
        return out, res
    return out
